# revision 1
# baseline (speedup 1.0000x reference)
"""AdaptiveRankingLoss distributed Bass kernel for 8 TRN2 NeuronCores.

Math
----
reference loss = sum_{i<j, t_i != t_j} w_ij * relu(margin_ij - sign(t_i - t_j)*(p_i - p_j))
                 / count,
  margin = 0.1 * clip(|t_i - t_j|, 0.1, 1.0),  w = 1/(1 + u_i + u_j).

The summand is symmetric under i<->j, and splitting by the sign of
a = t_j - t_i gives an exactly equivalent full-matrix form with no sign(),
no abs() and no triangular mask:

    numerator = sum_{all i,j} [a_ij > 0] * w_ij * relu(clip(0.1*a_ij, .01, .1) - (p_j - p_i))

Ties (a == 0, including the diagonal) contribute exactly 0 via the
indicator, and `count` is computed exactly on the host from duplicate
analysis of t.

Device mapping (per core: 1024 rows x 8192 cols of the pair matrix)
------------------------------------------------------------------
* one custom 8-stage DVE op produces v = [a>0]*relu(clip(0.1a,.01,.1)-b)
  per element (fp32 internal, bf16 out), streaming the broadcast column
  vectors with the row values as per-partition scalars.
* the weight w = 1/(1+u_i+u_j) is applied through a degree-6 bilinear
  polynomial 1/(2+z) ~ p(z), z = x_i + x_j, x = u - 0.5:
      w_ij ~ sum_n Phi_n(x_i) * x_j^n
  so  sum_ij v_ij w_ij = sum_{n,j} X[n,j] * Psi[n,j]  with
      X[n,j] = sum_i Phi_n(x_i) v_ij   (TensorEngine matmul, PSUM accum)
      Psi[n,j] = x_j^n.
* drains: X is staged out of PSUM by the (otherwise idle) scalar engine,
  partition-reshaped to [112, 256] by DMA, and contracted against Psi by a
  fused custom multiply-reduce; the tail piece reads PSUM directly. The
  host sums the per-core accumulators and divides by the exact pair count.

Host-side marshalling: inputs are sorted by target (the loss is
permutation-invariant) with rows strided across cores, so the [a>0]
indicator becomes triangular and whole column ranges are provably zero
and skipped (bit-exact). Columns are pre-scaled by 0.1 and cast to fp16.
"""

import numpy as np

import concourse.bass as bass
import concourse.bacc as bacc
import concourse.mybir as mybir
import concourse.tile as tile
from concourse.bass_utils import run_bass_kernel_spmd
from concourse import dve_ops
from concourse.dve_spec import (
    Spec,
    Src0,
    Src1,
    C0,
    C1,
    C2,
    Zero,
    relu,
    maxx,
    minn,
    lower,
    _has_src1,
)
from concourse.dve_uop import DveOpSpec

F32 = mybir.dt.float32
BF16 = mybir.dt.bfloat16

N = 8192          # problem size (hardcoded per spec)
NCORES = 8
P = 128           # SBUF partitions
R = N // NCORES   # rows per core (1024)
RT = R // P       # row tiles per core (8)
FC = 1024         # column chunk
NCH = N // FC     # chunks (8)
DEG = 6           # weight polynomial degree
K = DEG + 1
MMF = 512         # matmul free-dim tile

# Inputs are sorted by target on the host and rows are strided across cores
# (core c gets sorted rows c, c+8, ...). Row-tile r of any core then covers
# sorted positions >= 1024*r, so column chunks c < r satisfy t_j <= t_i
# everywhere -> the [a>0] indicator is identically 0 and the chunk is skipped
# for that tile. Bit-exact with the unskipped computation.


# --------------------------------------------------------------------------
# custom DVE op: v = [Src0 - C0 > 0] * relu(clip(Src0 - C0, C2^2, C2) - (Src1 - C1))
# Src0 = 0.1*t_col, C0 = 0.1*t_row, Src1 = p_col, C1 = p_row, C2 = 0.1.
# --------------------------------------------------------------------------
_ARL_NAME = "ARL_MAIN_V1"


def _arl_reference(in0, in1, s0, s1, imm2):
    a = in0 - s0
    m = np.clip(a, np.float32(imm2) * np.float32(imm2), imm2)
    return (a > 0).astype(np.float32) * np.maximum(m - (in1 - s1), 0.0)


def _register_arl_op():
    for op in dve_ops.OPS:
        if op.name == _ARL_NAME:
            return op
    a = Src0 - C0
    m = minn(maxx(a, C2 * C2), C2)
    h = relu(m - (Src1 - C1))
    spec = Spec(body=(a > Zero) * h, reference=_arl_reference)
    row = dve_ops._CUSTOM_DVE_ROW_BASE + len(dve_ops.OPS)
    assert row < 0x20, "custom-DVE row overflow"
    dve_ops._SUB_OPCODE_FOR_NAME[_ARL_NAME] = row
    shas = {}
    for ver in ("v3", "v4"):
        try:
            uops = lower(spec, ver=ver)
            shas[ver] = DveOpSpec(
                name=_ARL_NAME, opcode=row, uops=uops, rd1_en=_has_src1(spec)
            ).sha(ver)
        except Exception:
            pass
    op = dve_ops.DveOp(_ARL_NAME, spec, subdim=False, uops_sha=shas)
    dve_ops.OPS.append(op)
    dve_ops.CUSTOM_DVE_SPECS[_ARL_NAME] = spec
    return op


ARL_MAIN = _register_arl_op()


# --------------------------------------------------------------------------
# degree-6 bilinear split of w = 1/(1+u_i+u_j) = 1/(2 + x_i + x_j), x = u-.5
# --------------------------------------------------------------------------
def _acoef_matrix() -> np.ndarray:
    from numpy.polynomial import chebyshev as _C
    from math import comb

    nodes = np.cos((2 * np.arange(DEG + 1) + 1) / (2 * (DEG + 1)) * np.pi)
    ch = _C.chebfit(nodes, 1.0 / (2.0 + nodes), DEG)
    c = _C.cheb2poly(ch)  # power-basis coeffs of p(z) ~ 1/(2+z) on [-1,1]
    A = np.zeros((K, K), np.float64)
    for mm in range(K):
        for nn in range(K):
            if mm + nn <= DEG:
                A[mm, nn] = c[mm + nn] * comb(mm + nn, mm)
    return A.astype(np.float32)


_ACOEF = _acoef_matrix()


# --------------------------------------------------------------------------
# device graph builder
# --------------------------------------------------------------------------
def _build_nc():
    from contextlib import ExitStack

    F16 = mybir.dt.float16
    HW = N // 2  # column half-width (4096)

    nc = bacc.Bacc(None, target_bir_lowering=False, debug=False)

    t01_ext = nc.declare_dram_parameter("t01col", [N], F16, isOutput=False)
    p_ext = nc.declare_dram_parameter("pcol", [N], F16, isOutput=False)
    u_ext = nc.declare_dram_parameter("ucol", [N], F32, isOutput=False)
    rows_ext = nc.declare_dram_parameter("rows3", [P, 3 * RT], F32, isOutput=False)
    a_ext = nc.declare_dram_parameter("acoef", [K, K], F32, isOutput=False)
    out_ext = nc.declare_dram_parameter("out", [448], F32, isOutput=True)

    with tile.TileContext(nc) as tc, ExitStack() as ctx:
        constp = ctx.enter_context(tc.tile_pool(name="const", bufs=1))
        colp = ctx.enter_context(tc.tile_pool(name="cols", bufs=1))
        vp = ctx.enter_context(tc.tile_pool(name="v", bufs=3))
        pp = ctx.enter_context(tc.tile_pool(name="psum", bufs=1, space="PSUM"))
        sp = ctx.enter_context(tc.tile_pool(name="small", bufs=1))
        dramp = ctx.enter_context(tc.tile_pool(name="dram", bufs=1, space="DRAM"))

        # ---- small prep DMAs; one fused row-scalar load + u/a coefs ----
        rows_sb = constp.tile([P, 3, RT], F32)
        nc.sync.dma_start(
            rows_sb[:], rows_ext[:, :].rearrange("p (s r) -> p s r", s=3)
        )
        t01row_sb = rows_sb[:, 0, :]
        prow_sb = rows_sb[:, 1, :]
        urow_sb = rows_sb[:, 2, :]
        # ---- full-width fp16 column tiles; ranges in processing order,
        # small prep loads slotted after the first two ----
        t01_sb = colp.tile([P, N], F16)
        p_sb = colp.tile([P, N], F16)

        def load_cols(lo, w):
            nc.sync.dma_start(
                t01_sb[:, lo : lo + w],
                bass.AP(tensor=t01_ext, offset=lo, ap=[[0, P], [1, w]]),
            )
            nc.sync.dma_start(
                p_sb[:, lo : lo + w],
                bass.AP(tensor=p_ext, offset=lo, ap=[[0, P], [1, w]]),
            )

        load_cols(7168, 1024)
        load_cols(6144, 1024)
        abuf = constp.tile([P, K, K], F32)
        a_src = bass.AP(tensor=a_ext, offset=0, ap=[[0, P], [K, K], [1, K]])
        nc.sync.dma_start(abuf[:], a_src)
        FB = N // P  # 64
        u64 = sp.tile([P, FB], F32)
        nc.sync.dma_start(u64[:], u_ext[:].rearrange("(p f) -> p f", p=P))
        load_cols(5120, 1024)
        load_cols(4096, 1024)
        load_cols(2048, 2048)
        load_cols(0, 2048)

        # ---- pairwise compute (see module docstring). The two smallest
        # upper-half row-tiles are emitted before the Phi/Psi prep so the
        # DVE starts the bulk work as soon as the tail columns land; prep
        # then fills the stream while the remaining columns load. ----
        Xh = {}

        def emit_main(half, cbase, r):
            c0 = max(cbase, r * 1024)
            w = cbase + HW - c0
            v = vp.tile([P, HW], BF16, tag="v", name=f"v{half}_{r}")
            nc.vector._custom_dve(
                ARL_MAIN,
                out=v[:, :w],
                in0=t01_sb[:, c0 : cbase + HW],
                in1=p_sb[:, c0 : cbase + HW],
                s0=t01row_sb[:, r : r + 1],
                s1=prow_sb[:, r : r + 1],
                imm2=0.1,
            )
            return v

        def emit_matmuls(half, cbase, tiles, r, v):
            c0 = max(cbase, r * 1024)
            w = cbase + HW - c0
            for s in range(w // MMF):
                gc = c0 + s * MMF
                top = min(gc // 1024, tiles - 1)
                if half == 0:
                    mm_start, mm_stop = (r == top), (r == 0)
                else:
                    mm_start, mm_stop = (r == 0), (r == top)
                nc.tensor.matmul(
                    Xh[half][:, gc - cbase : gc - cbase + MMF],
                    phib[:, r, :],
                    v[:, s * MMF : (s + 1) * MMF],
                    start=mm_start,
                    stop=mm_stop,
                )

        def emit_group(half, cbase, tiles, r):
            v = emit_main(half, cbase, r)
            emit_matmuls(half, cbase, tiles, r, v)

        Xh[0] = pp.tile([K, HW], F32, tag="X", name="X0u")
        v7 = emit_main(0, HW, RT - 1)
        v6 = emit_main(0, HW, RT - 2)

        # ---- Phi[p, r, n] = sum_m A[m, n] * x_row^m (Horner), bf16 ----
        xrow = sp.tile([P, RT], F32)
        nc.vector.tensor_scalar_sub(xrow[:], urow_sb[:], 0.5)
        phit = sp.tile([P, RT, K], F32)
        nc.vector.tensor_copy(
            phit[:], abuf[:, DEG : DEG + 1, :].broadcast_to([P, RT, K])
        )
        xrow_b = xrow[:, :, None].broadcast_to([P, RT, K])
        for m in range(DEG - 1, -1, -1):
            nc.vector.tensor_mul(phit[:], phit[:], xrow_b)
            nc.vector.tensor_add(
                phit[:], phit[:], abuf[:, m : m + 1, :].broadcast_to([P, RT, K])
            )
        phib = constp.tile([P, RT, K], BF16)
        nc.vector.tensor_copy(phib[:], phit[:])

        # ---- Psi[n, j] = x_j^n via [P, 64] layout + DRAM bounce ----
        psi_dram = dramp.tile([K, N], F32)
        x64 = sp.tile([P, FB], F32)
        nc.vector.tensor_scalar_sub(x64[:], u64[:], 0.5)
        ones64 = sp.tile([P, FB], F32)
        nc.vector.memset(ones64[:], 1.0)
        nc.sync.dma_start(psi_dram[0, :].rearrange("(p f) -> p f", p=P), ones64[:])
        nc.sync.dma_start(psi_dram[1, :].rearrange("(p f) -> p f", p=P), x64[:])
        prev = x64
        for n in range(2, K):
            nxt = sp.tile([P, FB], F32, tag=f"pw{n}")
            nc.vector.tensor_mul(nxt[:], prev[:], x64[:])
            nc.sync.dma_start(psi_dram[n, :].rearrange("(p f) -> p f", p=P), nxt[:])
            prev = nxt
        # reshaped Psi for the overlapped drains; direct slices for the two
        # small trailing pieces
        psiR1 = constp.tile([K * 16, HW // 16], F32)
        psiR0a = constp.tile([K * 16, HW // 32], F32)
        for n in range(K):
            nc.sync.dma_start(
                psiR1[n * 16 : (n + 1) * 16, :],
                psi_dram[n : n + 1, HW:].rearrange("o (k f) -> o k f", k=16),
            )
            nc.sync.dma_start(
                psiR0a[n * 16 : (n + 1) * 16, :],
                psi_dram[n : n + 1, 0 : HW // 2].rearrange("o (k f) -> o k f", k=16),
            )
        psi0b = constp.tile([K, HW // 2], F32)
        nc.sync.dma_start(psi0b[:], psi_dram[:, HW // 2 : HW])

        accP = sp.tile([K * 16, 4], F32)
        nc.vector.memset(accP[:], 0.0)
        accA = accP[:, 0:1]
        accB = accP[:, 1:2]
        accC = accP[:K, 2:3]
        accD = accP[:K, 3:4]
        ttr_scr = sp.tile([K * 16, HW // 16], F32)
        ttr_scr2 = sp.tile([K, HW // 2], F32)



        emit_matmuls(0, HW, RT, RT - 1, v7)
        emit_matmuls(0, HW, RT, RT - 2, v6)
        for r in range(RT - 3, -1, -1):
            emit_group(0, HW, RT, r)
        Xsb1 = sp.tile([K, HW], F32)
        nc.scalar.copy(Xsb1[:], Xh[0][:])
        xr1 = sp.tile([K * 16, HW // 16], F32)
        for n in range(K):
            nc.sync.dma_start(
                xr1[n * 16 : (n + 1) * 16, :],
                Xsb1[n : n + 1, :].rearrange("o (k f) -> o k f", k=16),
            )
        Xh[1] = pp.tile([K, HW], F32, tag="X", name="X1l")
        for r in range(RT // 2):
            emit_group(1, 0, RT // 2, r)
        # drains: overlapped pieces go through ACT-copy + partition-reshape
        # DMA + a full-width mini-reduce; the tail piece reads PSUM directly.
        Xsb0a = sp.tile([K, HW // 2], F32)
        nc.scalar.copy(Xsb0a[:], Xh[1][:, 0 : HW // 2])
        xr0a = sp.tile([K * 16, HW // 32], F32)
        for n in range(K):
            nc.sync.dma_start(
                xr0a[n * 16 : (n + 1) * 16, :],
                Xsb0a[n : n + 1, :].rearrange("o (k f) -> o k f", k=16),
            )
        nc.vector._custom_dve(
            dve_ops.TENSOR_TENSOR_REDUCE,
            out=ttr_scr[:],
            in0=xr1[:],
            in1=psiR1[:],
            s0=0.0,
            s1=1.0,
            accum_out=accA,
        )
        nc.vector._custom_dve(
            dve_ops.TENSOR_TENSOR_REDUCE,
            out=ttr_scr[:, : HW // 32],
            in0=xr0a[:],
            in1=psiR0a[:],
            s0=0.0,
            s1=1.0,
            accum_out=accB,
        )
        nc.vector._custom_dve(
            dve_ops.TENSOR_TENSOR_REDUCE,
            out=ttr_scr2[:, :1024],
            in0=Xh[1][:, HW // 2 : HW // 2 + 1024],
            in1=psi0b[:, :1024],
            s0=0.0,
            s1=1.0,
            accum_out=accC,
        )
        nc.vector._custom_dve(
            dve_ops.TENSOR_TENSOR_REDUCE,
            out=ttr_scr2[:, :1024],
            in0=Xh[1][:, HW // 2 + 1024 : HW],
            in1=psi0b[:, 1024:],
            s0=0.0,
            s1=1.0,
            accum_out=accD,
        )
        nc.sync.dma_start(
            out_ext[0 : 4 * 112].rearrange("(p c) -> p c", c=4), accP[:]
        )

    nc.compile()
    return nc


_NC_CACHE = None


def _get_nc():
    global _NC_CACHE
    if _NC_CACHE is None:
        _NC_CACHE = _build_nc()
    return _NC_CACHE


def _exact_count(t: np.ndarray) -> int:
    n = t.shape[0]
    _, cnts = np.unique(t, return_counts=True)
    dup = int(sum(int(c) * (int(c) - 1) // 2 for c in cnts[cnts > 1]))
    return n * (n - 1) // 2 - dup


def _make_in_maps(predictions, targets, uncertainties):
    t = np.ascontiguousarray(np.asarray(targets, np.float32))
    p = np.ascontiguousarray(np.asarray(predictions, np.float32))
    u = np.ascontiguousarray(np.asarray(uncertainties, np.float32))
    # sort by target (loss is permutation invariant); stride rows across
    # cores so every core sees the same triangular-skip schedule.
    order = np.argsort(t, kind="stable")
    ts, ps, us = t[order], p[order], u[order]
    t01 = (np.float32(0.1) * ts).astype(np.float32)
    t01_h = t01.astype(np.float16)
    ps_h = ps.astype(np.float16)
    in_maps = []
    for i in range(NCORES):
        in_maps.append(
            {
                "t01col": t01_h,
                "pcol": ps_h,
                "ucol": us,
                "rows3": np.ascontiguousarray(
                    np.stack([t01[i::NCORES], ps[i::NCORES], us[i::NCORES]])
                    .reshape(3, RT, P)
                    .transpose(2, 0, 1)
                    .reshape(P, 3 * RT)
                ),
                "acoef": _ACOEF,
            }
        )
    return in_maps, t


def _run_device(in_maps, trace=False, **kw):
    nc = _get_nc()
    return run_bass_kernel_spmd(
        nc, in_maps, core_ids=list(range(NCORES)), trace=trace, **kw
    )


def kernel(predictions, targets, uncertainties):
    in_maps, t = _make_in_maps(predictions, targets, uncertainties)
    res = _run_device(in_maps)
    total = np.float64(0.0)
    for r in res.results:
        total += np.asarray(r["out"], np.float64).sum()
    count = _exact_count(t)
    return np.asarray(total / max(count, 1), dtype=np.float32)



# revision 4
# speedup vs baseline: 1.2366x; 1.2366x over previous
"""AdaptiveRankingLoss distributed Bass kernel for 8 TRN2 NeuronCores (v2).

Math
----
loss = sum_{i<j, t_i != t_j} w_ij * relu(m_ij - sign(t_i-t_j)*(p_i-p_j)) / count
  m = 0.1*clip(|t_i-t_j|, 0.1, 1.0),  w = 1/(1+u_i+u_j).

Host sorts by t; with a = t_j - t_i (sorted ascending) the full-matrix
summand [a>0] * w * relu(clip(0.1a, .01, .1) - (p_j - p_i)) is split into
t-distance BANDS with cheap per-band formulas:

  far  (a >= 1):      relu(0.1  - (p_j - p_i)) = relu(pn_j + (p_i + 0.1))
  mid  (0.1<=a<=1):   relu(0.1a - (p_j - p_i)) = relu(q_j  + (-q_i))
  near (0<a<0.1):     relu(0.01 - (p_j - p_i)) = relu(pn_j + (p_i + 0.01))
  mixed/indicator:    full custom 8-stage DVE op
with pn = -p, q = 0.1t - p (both fp16 broadcast columns). Pure-band runs
use plain TENSOR_SCALAR (add, max0) with per-partition fp32 scalars at
~3.7x the custom op's throughput; a few large far runs go to the (idle)
scalar/ACT engine as relu(1*pn + bias).

Band boundaries are band-uniform (computed from each 1024-row band's
t-range) so the single SPMD program is valid for every core: core c owns
rows {1024k + 8p + c} (strided within band), and each band's zone edges
cover all cores' rows.

Weights via the degree-6 bilinear split  w ~ sum_n Phi_n(x_i) Psi_n(x_j)
(x = u - 0.5): v-tiles are contracted over the 128 row-partitions by the
TensorEngine into X[n, j] (PSUM, 16 x [7,512] chunks over 8 banks, two
column phases), then sum_j X[n,j] Psi_n(j) via DVE tensor-tensor-reduce,
mostly on a DMA-partition-reshaped [112, .] layout. Host sums the 8
cores' accumulators and divides by the exact (tie-aware) pair count.
"""

import numpy as np

import concourse.bass as bass
import concourse.bacc as bacc
import concourse.mybir as mybir
import concourse.tile as tile
from concourse.bass_utils import run_bass_kernel_spmd
from concourse import dve_ops
from concourse.dve_spec import (
    Spec,
    Src0,
    Src1,
    C0,
    C1,
    C2,
    Zero,
    relu,
    maxx,
    minn,
    lower,
    _has_src1,
)
from concourse.dve_uop import DveOpSpec

F32 = mybir.dt.float32
F16 = mybir.dt.float16
BF16 = mybir.dt.bfloat16
AL = mybir.AluOpType

N = 8192
NCORES = 8
P = 128
NSLOT = 8          # 1024-row bands
K = 7              # weight polynomial terms
CH = 512           # psum chunk width
NCHUNK = N // CH   # 16
PHASE_SPLIT = 4096
MIN_RUN = 384      # pure runs narrower than this merge into custom


# --------------------------------------------------------------------------
# custom DVE op with NEGATED p column:
#   a = Src0 - C0;  v = [a>0] * relu(clip(a, C2^2, C2) + Src1 + C1)
#   Src0 = 0.1*t_col (f16), C0 = 0.1*t_row, Src1 = -p_col (f16), C1 = p_row.
# --------------------------------------------------------------------------
_ARL_NAME = "ARL_PN_V2"


def _arl_reference(in0, in1, s0, s1, imm2):
    a = in0 - s0
    m = np.clip(a, np.float32(imm2) * np.float32(imm2), imm2)
    return (a > 0).astype(np.float32) * np.maximum(m + in1 + s1, 0.0)


def _register_arl_op():
    for op in dve_ops.OPS:
        if op.name == _ARL_NAME:
            return op
    a = Src0 - C0
    m = minn(maxx(a, C2 * C2), C2)
    h = relu((m + Src1) + C1)
    spec = Spec(body=(a > Zero) * h, reference=_arl_reference)
    row = dve_ops._CUSTOM_DVE_ROW_BASE + len(dve_ops.OPS)
    assert row < 0x20, "custom-DVE row overflow"
    dve_ops._SUB_OPCODE_FOR_NAME[_ARL_NAME] = row
    shas = {}
    for ver in ("v3", "v4"):
        try:
            uops = lower(spec, ver=ver)
            shas[ver] = DveOpSpec(
                name=_ARL_NAME, opcode=row, uops=uops, rd1_en=_has_src1(spec)
            ).sha(ver)
        except Exception:
            pass
    op = dve_ops.DveOp(_ARL_NAME, spec, subdim=False, uops_sha=shas)
    dve_ops.OPS.append(op)
    dve_ops.CUSTOM_DVE_SPECS[_ARL_NAME] = spec
    return op


ARL_PN = _register_arl_op()


# --------------------------------------------------------------------------
# degree-6 bilinear split of w = 1/(1+u_i+u_j) = 1/(2 + x_i + x_j), x = u-.5
# --------------------------------------------------------------------------
def _acoef_matrix() -> np.ndarray:
    from numpy.polynomial import chebyshev as _C
    from math import comb

    deg = K - 1
    nodes = np.cos((2 * np.arange(deg + 1) + 1) / (2 * (deg + 1)) * np.pi)
    ch = _C.chebfit(nodes, 1.0 / (2.0 + nodes), deg)
    c = _C.cheb2poly(ch)
    A = np.zeros((K, K), np.float64)
    for mm in range(K):
        for nn in range(K):
            if mm + nn <= deg:
                A[mm, nn] = c[mm + nn] * comb(mm + nn, mm)
    return A  # float64


_ACOEF = _acoef_matrix()


# --------------------------------------------------------------------------
# plan: band zones + run lists from the sorted targets (host side)
# --------------------------------------------------------------------------
def _make_plan(ts32: np.ndarray):
    ts = ts32.astype(np.float64)
    slot_runs = []  # per slot: list of (kind, a, b) over full column range
    for k in range(NSLOT):
        t_lo = ts[1024 * k]
        t_hi = ts[1024 * k + 1023]
        jA = 1024 * k  # extend down to band start (extra cols give exact 0)
        B1 = int(np.searchsorted(ts, t_hi + 0.1, "left"))
        B2 = int(np.searchsorted(ts, t_lo + 1.0, "right"))
        B3 = int(np.searchsorted(ts, t_hi + 1.0, "left"))
        B1 = max(B1, jA)
        B2 = max(B2, B1)
        B3 = max(B3, B2)
        runs = []
        if B1 > jA:
            runs.append(["cust", jA, B1])
        if B2 > B1:
            runs.append(["mid", B1, B2])
        if B3 > B2:
            runs.append(["cust", B2, B3])
        if N > B3:
            runs.append(["far", B3, N])
        # merge tiny pure runs into custom neighbours
        changed = True
        while changed:
            changed = False
            for i, r in enumerate(runs):
                if r[0] != "cust" and r[2] - r[1] < MIN_RUN:
                    r[0] = "cust"
                    changed = True
            i = 0
            while i + 1 < len(runs):
                if runs[i][0] == "cust" and runs[i + 1][0] == "cust":
                    runs[i][2] = runs[i + 1][2]
                    del runs[i + 1]
                    changed = True
                else:
                    i += 1
        slot_runs.append([(r[0], r[1], r[2]) for r in runs])

    # phase-clipped emissions
    def clip_runs(runs, lo, hi):
        out = []
        for kind, a, b in runs:
            a2, b2 = max(a, lo), min(b, hi)
            if b2 > a2:
                out.append((kind, a2, b2))
        return out

    emitA = [clip_runs(slot_runs[k], PHASE_SPLIT, N) for k in range(NSLOT)]
    emitB = [clip_runs(slot_runs[k], 0, PHASE_SPLIT) for k in range(NSLOT)]

    # split slot0's phase-A far run for a faster PE start
    if emitA[0] and emitA[0][0][0] == "far" and emitA[0][0][2] - emitA[0][0][1] > 2048:
        kind, a, b = emitA[0][0]
        m1 = a + 1024
        m2 = a + 2048
        emitA[0] = [(kind, a, m1), (kind, m1, m2), (kind, m2, b)] + emitA[0][1:]

    # chunk first/last slot tables per phase
    def chunk_tables(emits, lo, hi):
        first = {}
        last = {}
        for k in range(NSLOT):
            for kind, a, b in emits[k]:
                c0, c1 = a // CH, (b - 1) // CH
                for c in range(c0, c1 + 1):
                    if c not in first:
                        first[c] = k
                    last[c] = k
        return first, last

    firstA, lastA = chunk_tables(emitA, PHASE_SPLIT, N)
    firstB, lastB = chunk_tables(emitB, 0, PHASE_SPLIT)

    # column ranges needed per tensor (rounded out to 256)
    def ranges_for(kinds):
        need = np.zeros(N, bool)
        for k in range(NSLOT):
            for kind, a, b in slot_runs[k]:
                if kind in kinds:
                    need[a:b] = True
        out = []
        j = 0
        while j < N:
            if need[j]:
                e = j
                while e < N and need[e]:
                    e += 1
                a = (j // 256) * 256
                b = min(N, ((e + 255) // 256) * 256)
                if out and a <= out[-1][1]:
                    out[-1] = (out[-1][0], b)
                else:
                    out.append((a, b))
                j = e
            else:
                j += 1
        return out

    t01_ranges = ranges_for(("cust",))
    q_ranges = ranges_for(("mid",))
    return {
        "emitA": emitA,
        "emitB": emitB,
        "firstA": firstA,
        "lastA": lastA,
        "firstB": firstB,
        "lastB": lastB,
        "t01_ranges": t01_ranges,
        "q_ranges": q_ranges,
    }


# --------------------------------------------------------------------------
# device graph
# --------------------------------------------------------------------------
# scalar slots in rows5: 0: 0.1*t  1: p  2: -q  3: p+0.01  4: p+0.1
_KIND_SCAL = {"mid": 2, "near": 3, "far": 4}
NACC = 8  # accumulator columns


def _build_nc(plan):
    from contextlib import ExitStack

    nc = bacc.Bacc(None, target_bir_lowering=False, debug=False)

    t01_ext = nc.declare_dram_parameter("t01col", [N], F16, isOutput=False)
    pn_ext = nc.declare_dram_parameter("pncol", [N], F16, isOutput=False)
    q_ext = nc.declare_dram_parameter("qcol", [N], F16, isOutput=False)
    rows_ext = nc.declare_dram_parameter("rows5", [P, 5 * NSLOT], F32, isOutput=False)
    phib_ext = nc.declare_dram_parameter("phib", [P, NSLOT * K], BF16, isOutput=False)
    psi_ext = nc.declare_dram_parameter("psi", [K, N], F32, isOutput=False)
    psiRA_ext = nc.declare_dram_parameter("psiRA", [K * 16, 256], F32, isOutput=False)
    psiRB_ext = nc.declare_dram_parameter("psiRB", [K * 16, 192], F32, isOutput=False)
    out_ext = nc.declare_dram_parameter("out", [K * 16 * NACC], F32, isOutput=True)

    emitA, emitB = plan["emitA"], plan["emitB"]
    firstA, lastA = plan["firstA"], plan["lastA"]
    firstB, lastB = plan["firstB"], plan["lastB"]

    with tile.TileContext(nc) as tc, ExitStack() as ctx:
        constp = ctx.enter_context(tc.tile_pool(name="const", bufs=1))
        colp = ctx.enter_context(tc.tile_pool(name="cols", bufs=1))
        vp = ctx.enter_context(tc.tile_pool(name="v", bufs=4))
        pp = ctx.enter_context(tc.tile_pool(name="psum", bufs=8, space="PSUM"))
        sp = ctx.enter_context(tc.tile_pool(name="small", bufs=1))

        t01_sb = colp.tile([P, N], F16)
        pn_sb = colp.tile([P, N], F16)
        q_sb = colp.tile([P, N], F16)

        def bcast(dst, ext, lo, hi):
            nc.sync.dma_start(
                dst[:, lo:hi],
                bass.AP(tensor=ext, offset=lo, ap=[[0, P], [1, hi - lo]]),
            )

        # phase-A column data first (pn upper half feeds the first runs)
        bcast(pn_sb, pn_ext, 4096, 6144)
        bcast(pn_sb, pn_ext, 6144, 8192)
        rows_sb = constp.tile([P, NSLOT, 5], F32)
        nc.sync.dma_start(
            rows_sb[:], rows_ext[:, :].rearrange("p (r s) -> p r s", s=5)
        )
        phib = constp.tile([P, NSLOT, K], BF16)
        nc.sync.dma_start(
            phib[:], phib_ext[:, :].rearrange("p (r k) -> p r k", k=K)
        )
        for lo, hi in plan["t01_ranges"]:
            if hi > 4096:
                bcast(t01_sb, t01_ext, max(lo, 4096), hi)
        for lo, hi in plan["q_ranges"]:
            if hi > 4096:
                bcast(q_sb, q_ext, max(lo, 4096), hi)
        bcast(pn_sb, pn_ext, 2048, 4096)
        bcast(pn_sb, pn_ext, 0, 2048)
        for lo, hi in plan["t01_ranges"]:
            if lo < 4096:
                bcast(t01_sb, t01_ext, lo, min(hi, 4096))
        for lo, hi in plan["q_ranges"]:
            if lo < 4096:
                bcast(q_sb, q_ext, lo, min(hi, 4096))
        psi = constp.tile([K, N], F32)
        nc.sync.dma_start(psi[:], psi_ext[:, :])
        psiRA = constp.tile([K * 16, 256], F32)
        nc.sync.dma_start(psiRA[:], psiRA_ext[:, :])
        psiRB = constp.tile([K * 16, 192], F32)
        nc.sync.dma_start(psiRB[:], psiRB_ext[:, :])

        acc = sp.tile([K * 16, NACC], F32)
        nc.vector.memset(acc[:], 0.0)
        XsA = sp.tile([K, 4096], F32)
        XsB = sp.tile([K, 3072], F32)
        ttr_scr = sp.tile([K * 16, 256], F32)
        ttr_scr2 = sp.tile([K, CH], F32)

        chunk_tiles = {}

        def get_chunk(c):
            if c not in chunk_tiles:
                chunk_tiles[c] = pp.tile([K, CH], F32, tag="X", name=f"X{c}")
            return chunk_tiles[c]

        # ACT offload set: phase-A far runs of slots 1..3
        act_far = set()
        for k in (1, 2, 3):
            for kind, a, b in emitA[k]:
                if kind == "far":
                    act_far.add((k, a, b))

        def emit_run(k, kind, a, b, first_t, last_t):
            while b - a > 4096:
                emit_run(k, kind, a, a + 4096, first_t, last_t)
                a += 4096
            w = b - a
            v = vp.tile([P, 4096], BF16, tag="v")
            if kind == "cust":
                nc.vector._custom_dve(
                    ARL_PN,
                    out=v[:, :w],
                    in0=t01_sb[:, a:b],
                    in1=pn_sb[:, a:b],
                    s0=rows_sb[:, k, 0:1],
                    s1=rows_sb[:, k, 1:2],
                    imm2=0.1,
                )
            elif (k, a, b) in act_far:
                nc.scalar.activation(
                    v[:, :w],
                    pn_sb[:, a:b],
                    mybir.ActivationFunctionType.Relu,
                    bias=rows_sb[:, k, 4:5],
                    scale=1.0,
                )
            else:
                src = q_sb if kind == "mid" else pn_sb
                nc.vector.tensor_scalar(
                    v[:, :w],
                    src[:, a:b],
                    rows_sb[:, k, _KIND_SCAL[kind] : _KIND_SCAL[kind] + 1],
                    0.0,
                    AL.add,
                    AL.max,
                )
            c0, c1 = a // CH, (b - 1) // CH
            for c in range(c0, c1 + 1):
                lo, hi = max(a, c * CH), min(b, (c + 1) * CH)
                nc.tensor.matmul(
                    get_chunk(c)[:, lo - c * CH : hi - c * CH],
                    phib[:, k, :],
                    v[:, lo - a : hi - a],
                    start=(k == first_t[c]),
                    stop=(k == last_t[c]),
                )

        def drain_copy(c, Xs, base):
            nc.scalar.copy(
                Xs[:, c * CH - base : (c + 1) * CH - base], get_chunk(c)[:]
            )

        def drain_direct(c, slot_idx):
            nc.vector._custom_dve(
                dve_ops.TENSOR_TENSOR_REDUCE,
                out=ttr_scr2[:],
                in0=get_chunk(c)[:],
                in1=psi[:, c * CH : (c + 1) * CH],
                s0=0.0,
                s1=1.0,
                accum_out=acc[0:K, slot_idx : slot_idx + 1],
            )

        # ---- phase A: columns [4096, 8192) ----
        drainedA = []
        for k in range(NSLOT):
            for kind, a, b in emitA[k]:
                emit_run(k, kind, a, b, firstA, lastA)
            for c in sorted(lastA):
                if lastA[c] == k:
                    drain_copy(c, XsA, PHASE_SPLIT)
                    drainedA.append(c)
        # reshape [7,4096] -> [112,256] (per-n DMAs); TTR emitted later so
        # the in-order DVE queue does not stall on the drain dependencies.
        xrA = sp.tile([K * 16, 256], F32)
        for n in range(K):
            nc.sync.dma_start(
                xrA[n * 16 : (n + 1) * 16, :],
                XsA[n : n + 1, :].rearrange("o (k f) -> o k f", k=16),
            )

        # ---- phase B: columns [0, 4096) ----
        xrB = sp.tile([K * 16, 192], F32)
        ttr_scrB = sp.tile([K * 16, 192], F32)
        for k in range(NSLOT):
            for kind, a, b in emitB[k]:
                emit_run(k, kind, a, b, firstB, lastB)
            for c in sorted(lastB):
                if lastB[c] == k:
                    if c <= 5:
                        drain_copy(c, XsB, 0)
                    else:
                        drain_direct(c, 2 + (c - 6))
            if k == 1:
                nc.vector._custom_dve(
                    dve_ops.TENSOR_TENSOR_REDUCE,
                    out=ttr_scr[:],
                    in0=xrA[:],
                    in1=psiRA[:],
                    s0=0.0,
                    s1=1.0,
                    accum_out=acc[:, 0:1],
                )
        for n in range(K):
            nc.sync.dma_start(
                xrB[n * 16 : (n + 1) * 16, :],
                XsB[n : n + 1, :].rearrange("o (k f) -> o k f", k=16),
            )
        nc.vector._custom_dve(
            dve_ops.TENSOR_TENSOR_REDUCE,
            out=ttr_scrB[:],
            in0=xrB[:],
            in1=psiRB[:],
            s0=0.0,
            s1=1.0,
            accum_out=acc[:, 1:2],
        )

        nc.sync.dma_start(
            out_ext[:].rearrange("(p c) -> p c", c=NACC), acc[:]
        )

    nc.compile()
    return nc


_NC_CACHE = {}


def _get_nc(plan, key):
    if key not in _NC_CACHE:
        _NC_CACHE[key] = _build_nc(plan)
    return _NC_CACHE[key]


def _exact_count(t: np.ndarray) -> int:
    n = t.shape[0]
    _, cnts = np.unique(t, return_counts=True)
    dup = int(sum(int(c) * (int(c) - 1) // 2 for c in cnts[cnts > 1]))
    return n * (n - 1) // 2 - dup


def _make_in_maps(predictions, targets, uncertainties):
    import ml_dtypes

    t = np.ascontiguousarray(np.asarray(targets, np.float32))
    p = np.ascontiguousarray(np.asarray(predictions, np.float32))
    u = np.ascontiguousarray(np.asarray(uncertainties, np.float32))
    order = np.argsort(t, kind="stable")
    ts, ps, us = t[order], p[order], u[order]
    ts64, ps64 = ts.astype(np.float64), ps.astype(np.float64)

    t01_h = (0.1 * ts64).astype(np.float16)
    pn_h = (-ps64).astype(np.float16)
    q_h = (0.1 * ts64 - ps64).astype(np.float16)

    # Psi[n, j] = x_j^n, x = u - 0.5
    x = us.astype(np.float64) - 0.5
    psi = np.stack([x**n for n in range(K)]).astype(np.float32)  # [K, N]
    psiRA = np.ascontiguousarray(
        psi[:, PHASE_SPLIT:].reshape(K, 16, 256).reshape(K * 16, 256)
    )
    psiRB = np.ascontiguousarray(
        psi[:, 0:3072].reshape(K, 16, 192).reshape(K * 16, 192)
    )

    # Phi[i, n] = sum_m A[m, n] x_i^m  (float64 -> bf16)
    xp = np.stack([x**m for m in range(K)])  # [K, N]
    phi = np.einsum("mn,mj->jn", _ACOEF, xp)  # [N, K]

    in_maps = []
    for c in range(NCORES):
        pos = (np.arange(NSLOT)[:, None] * 1024 + 8 * np.arange(P)[None, :] + c)
        # rows5[p, slot, s]
        rows5 = np.zeros((P, NSLOT, 5), np.float64)
        tp = ts64[pos]  # [slot, p]
        pp_ = ps64[pos]
        rows5[:, :, 0] = (0.1 * tp).T
        rows5[:, :, 1] = pp_.T
        rows5[:, :, 2] = (-(0.1 * tp - pp_)).T
        rows5[:, :, 3] = (pp_ + 0.01).T
        rows5[:, :, 4] = (pp_ + 0.1).T
        phib = phi[pos, :]  # [slot, p, K]
        phib = np.ascontiguousarray(np.transpose(phib, (1, 0, 2)))  # [p, slot, K]
        in_maps.append(
            {
                "t01col": t01_h,
                "pncol": pn_h,
                "qcol": q_h,
                "rows5": np.ascontiguousarray(
                    rows5.reshape(P, 5 * NSLOT)
                ).astype(np.float32),
                "phib": phib.reshape(P, NSLOT * K).astype(ml_dtypes.bfloat16),
                "psi": psi,
                "psiRA": psiRA,
                "psiRB": psiRB,
            }
        )
    plan = _make_plan(ts)
    return in_maps, t, plan


def _run_device(in_maps, plan, plan_key, trace=False, **kw):
    nc = _get_nc(plan, plan_key)
    return run_bass_kernel_spmd(
        nc, in_maps, core_ids=list(range(NCORES)), trace=trace, **kw
    )


def _plan_key(plan):
    return (
        tuple(tuple(r) for k in range(NSLOT) for r in plan["emitA"][k]),
        tuple(tuple(r) for k in range(NSLOT) for r in plan["emitB"][k]),
    )


def kernel(predictions, targets, uncertainties):
    in_maps, t, plan = _make_in_maps(predictions, targets, uncertainties)
    res = _run_device(in_maps, plan, _plan_key(plan))
    total = np.float64(0.0)
    for r in res.results:
        total += np.asarray(r["out"], np.float64).sum()
    count = _exact_count(t)
    return np.asarray(total / max(count, 1), dtype=np.float32)


# revision 6
# speedup vs baseline: 1.2513x; 1.0118x over previous
"""AdaptiveRankingLoss distributed Bass kernel for 8 TRN2 NeuronCores (v2).

Math
----
loss = sum_{i<j, t_i != t_j} w_ij * relu(m_ij - sign(t_i-t_j)*(p_i-p_j)) / count
  m = 0.1*clip(|t_i-t_j|, 0.1, 1.0),  w = 1/(1+u_i+u_j).

Host sorts by t; with a = t_j - t_i (sorted ascending) the full-matrix
summand [a>0] * w * relu(clip(0.1a, .01, .1) - (p_j - p_i)) is split into
t-distance BANDS with cheap per-band formulas:

  far  (a >= 1):      relu(0.1  - (p_j - p_i)) = relu(pn_j + (p_i + 0.1))
  mid  (0.1<=a<=1):   relu(0.1a - (p_j - p_i)) = relu(q_j  + (-q_i))
  near (0<a<0.1):     relu(0.01 - (p_j - p_i)) = relu(pn_j + (p_i + 0.01))
  mixed/indicator:    full custom 8-stage DVE op
with pn = -p, q = 0.1t - p (both fp16 broadcast columns). Pure-band runs
use plain TENSOR_SCALAR (add, max0) with per-partition fp32 scalars at
~3.7x the custom op's throughput; a few large far runs go to the (idle)
scalar/ACT engine as relu(1*pn + bias).

Band boundaries are band-uniform (computed from each 1024-row band's
t-range) so the single SPMD program is valid for every core: core c owns
rows {1024k + 8p + c} (strided within band), and each band's zone edges
cover all cores' rows.

Weights via the degree-6 bilinear split  w ~ sum_n Phi_n(x_i) Psi_n(x_j)
(x = u - 0.5): v-tiles are contracted over the 128 row-partitions by the
TensorEngine into X[n, j] (PSUM, 16 x [7,512] chunks over 8 banks, two
column phases), then sum_j X[n,j] Psi_n(j) via DVE tensor-tensor-reduce,
mostly on a DMA-partition-reshaped [112, .] layout. Host sums the 8
cores' accumulators and divides by the exact (tie-aware) pair count.
"""

import numpy as np

import concourse.bass as bass
import concourse.bacc as bacc
import concourse.mybir as mybir
import concourse.tile as tile
from concourse.bass_utils import run_bass_kernel_spmd
from concourse import dve_ops
from concourse.dve_spec import (
    Spec,
    Src0,
    Src1,
    C0,
    C1,
    C2,
    Zero,
    relu,
    maxx,
    minn,
    lower,
    _has_src1,
)
from concourse.dve_uop import DveOpSpec

F32 = mybir.dt.float32
F16 = mybir.dt.float16
BF16 = mybir.dt.bfloat16
AL = mybir.AluOpType

N = 8192
NCORES = 8
P = 128
NSLOT = 8          # 1024-row bands
K = 7              # weight polynomial terms
CH = 1024          # psum chunk width (2 banks)
NCHUNK = N // CH   # 8
PHASE_SPLIT = 4096
MIN_RUN = 384      # pure runs narrower than this merge into custom


# --------------------------------------------------------------------------
# custom DVE op with NEGATED p column:
#   a = Src0 - C0;  v = [a>0] * relu(clip(a, C2^2, C2) + Src1 + C1)
#   Src0 = 0.1*t_col (f16), C0 = 0.1*t_row, Src1 = -p_col (f16), C1 = p_row.
# --------------------------------------------------------------------------
_ARL_NAME = "ARL_PN_V2"


def _arl_reference(in0, in1, s0, s1, imm2):
    a = in0 - s0
    m = np.clip(a, np.float32(imm2) * np.float32(imm2), imm2)
    return (a > 0).astype(np.float32) * np.maximum(m + in1 + s1, 0.0)


def _register_arl_op():
    for op in dve_ops.OPS:
        if op.name == _ARL_NAME:
            return op
    a = Src0 - C0
    m = minn(maxx(a, C2 * C2), C2)
    h = relu((m + Src1) + C1)
    spec = Spec(body=(a > Zero) * h, reference=_arl_reference)
    row = dve_ops._CUSTOM_DVE_ROW_BASE + len(dve_ops.OPS)
    assert row < 0x20, "custom-DVE row overflow"
    dve_ops._SUB_OPCODE_FOR_NAME[_ARL_NAME] = row
    shas = {}
    for ver in ("v3", "v4"):
        try:
            uops = lower(spec, ver=ver)
            shas[ver] = DveOpSpec(
                name=_ARL_NAME, opcode=row, uops=uops, rd1_en=_has_src1(spec)
            ).sha(ver)
        except Exception:
            pass
    op = dve_ops.DveOp(_ARL_NAME, spec, subdim=False, uops_sha=shas)
    dve_ops.OPS.append(op)
    dve_ops.CUSTOM_DVE_SPECS[_ARL_NAME] = spec
    return op


ARL_PN = _register_arl_op()


# --------------------------------------------------------------------------
# degree-6 bilinear split of w = 1/(1+u_i+u_j) = 1/(2 + x_i + x_j), x = u-.5
# --------------------------------------------------------------------------
def _acoef_matrix() -> np.ndarray:
    from numpy.polynomial import chebyshev as _C
    from math import comb

    deg = K - 1
    nodes = np.cos((2 * np.arange(deg + 1) + 1) / (2 * (deg + 1)) * np.pi)
    ch = _C.chebfit(nodes, 1.0 / (2.0 + nodes), deg)
    c = _C.cheb2poly(ch)
    A = np.zeros((K, K), np.float64)
    for mm in range(K):
        for nn in range(K):
            if mm + nn <= deg:
                A[mm, nn] = c[mm + nn] * comb(mm + nn, mm)
    return A  # float64


_ACOEF = _acoef_matrix()


# --------------------------------------------------------------------------
# plan: band zones + run lists from the sorted targets (host side)
# --------------------------------------------------------------------------
def _make_plan(ts32: np.ndarray):
    ts = ts32.astype(np.float64)
    slot_runs = []  # per slot: list of (kind, a, b) over full column range
    for k in range(NSLOT):
        t_lo = ts[1024 * k]
        t_hi = ts[1024 * k + 1023]
        jA = 1024 * k  # extend down to band start (extra cols give exact 0)
        B1 = int(np.searchsorted(ts, t_hi + 0.1, "left"))
        B2 = int(np.searchsorted(ts, t_lo + 1.0, "right"))
        B3 = int(np.searchsorted(ts, t_hi + 1.0, "left"))
        B1 = max(B1, jA)
        B2 = max(B2, B1)
        B3 = max(B3, B2)
        runs = []
        if B1 > jA:
            runs.append(["cust", jA, B1])
        if B2 > B1:
            runs.append(["mid", B1, B2])
        if B3 > B2:
            runs.append(["cust", B2, B3])
        if N > B3:
            runs.append(["far", B3, N])
        # merge tiny pure runs into custom neighbours
        changed = True
        while changed:
            changed = False
            for i, r in enumerate(runs):
                if r[0] != "cust" and r[2] - r[1] < MIN_RUN:
                    r[0] = "cust"
                    changed = True
            i = 0
            while i + 1 < len(runs):
                if runs[i][0] == "cust" and runs[i + 1][0] == "cust":
                    runs[i][2] = runs[i + 1][2]
                    del runs[i + 1]
                    changed = True
                else:
                    i += 1
        slot_runs.append([(r[0], r[1], r[2]) for r in runs])

    # phase-clipped emissions
    def clip_runs(runs, lo, hi):
        out = []
        for kind, a, b in runs:
            a2, b2 = max(a, lo), min(b, hi)
            if b2 > a2:
                out.append((kind, a2, b2))
        return out

    emitA = [clip_runs(slot_runs[k], PHASE_SPLIT, N) for k in range(NSLOT)]
    emitB = [clip_runs(slot_runs[k], 0, PHASE_SPLIT) for k in range(NSLOT)]

    # split slot0's phase-A far run for a faster PE start
    if emitA[0] and emitA[0][0][0] == "far" and emitA[0][0][2] - emitA[0][0][1] > 2048:
        kind, a, b = emitA[0][0]
        cuts = [a, a + 512, a + 1024, a + 2048, b]
        emitA[0] = [(kind, x, y) for x, y in zip(cuts, cuts[1:])] + emitA[0][1:]

    # chunk first/last slot tables per phase
    def chunk_tables(emits, lo, hi):
        first = {}
        last = {}
        for k in range(NSLOT):
            for kind, a, b in emits[k]:
                c0, c1 = a // CH, (b - 1) // CH
                for c in range(c0, c1 + 1):
                    if c not in first:
                        first[c] = k
                    last[c] = k
        return first, last

    firstA, lastA = chunk_tables(emitA, PHASE_SPLIT, N)
    firstB, lastB = chunk_tables(emitB, 0, PHASE_SPLIT)

    # column ranges needed per tensor (rounded out to 256)
    def ranges_for(kinds):
        need = np.zeros(N, bool)
        for k in range(NSLOT):
            for kind, a, b in slot_runs[k]:
                if kind in kinds:
                    need[a:b] = True
        out = []
        j = 0
        while j < N:
            if need[j]:
                e = j
                while e < N and need[e]:
                    e += 1
                a = (j // 256) * 256
                b = min(N, ((e + 255) // 256) * 256)
                if out and a <= out[-1][1]:
                    out[-1] = (out[-1][0], b)
                else:
                    out.append((a, b))
                j = e
            else:
                j += 1
        return out

    t01_ranges = ranges_for(("cust",))
    q_ranges = ranges_for(("mid",))
    return {
        "emitA": emitA,
        "emitB": emitB,
        "firstA": firstA,
        "lastA": lastA,
        "firstB": firstB,
        "lastB": lastB,
        "t01_ranges": t01_ranges,
        "q_ranges": q_ranges,
    }


# --------------------------------------------------------------------------
# device graph
# --------------------------------------------------------------------------
# scalar slots in rows5: 0: 0.1*t  1: p  2: -q  3: p+0.01  4: p+0.1
_KIND_SCAL = {"mid": 2, "near": 3, "far": 4}
NACC = 8  # accumulator columns


def _build_nc(plan):
    from contextlib import ExitStack

    nc = bacc.Bacc(None, target_bir_lowering=False, debug=False)

    t01_ext = nc.declare_dram_parameter("t01col", [N], F16, isOutput=False)
    pn_ext = nc.declare_dram_parameter("pncol", [N], F16, isOutput=False)
    q_ext = nc.declare_dram_parameter("qcol", [N], F16, isOutput=False)
    rows_ext = nc.declare_dram_parameter("rows5", [P, 5 * NSLOT], F32, isOutput=False)
    phib_ext = nc.declare_dram_parameter("phib", [P, NSLOT * K], BF16, isOutput=False)
    psi_ext = nc.declare_dram_parameter("psi", [K, N], F32, isOutput=False)
    psiRA_ext = nc.declare_dram_parameter("psiRA", [K * 16, 256], F32, isOutput=False)
    psiRB_ext = nc.declare_dram_parameter("psiRB", [K * 16, 128], F32, isOutput=False)
    out_ext = nc.declare_dram_parameter("out", [K * 16 * NACC], F32, isOutput=True)

    emitA, emitB = plan["emitA"], plan["emitB"]
    firstA, lastA = plan["firstA"], plan["lastA"]
    firstB, lastB = plan["firstB"], plan["lastB"]

    with tile.TileContext(nc) as tc, ExitStack() as ctx:
        constp = ctx.enter_context(tc.tile_pool(name="const", bufs=1))
        colp = ctx.enter_context(tc.tile_pool(name="cols", bufs=1))
        vp = ctx.enter_context(tc.tile_pool(name="v", bufs=4))
        pp = ctx.enter_context(tc.tile_pool(name="psum", bufs=4, space="PSUM"))
        sp = ctx.enter_context(tc.tile_pool(name="small", bufs=1))

        t01_sb = colp.tile([P, N], F16)
        pn_sb = colp.tile([P, N], F16)
        q_sb = colp.tile([P, N], F16)

        def bcast(dst, ext, lo, hi, piece=1024):
            while lo < hi:
                m = min(lo + piece, hi)
                nc.sync.dma_start(
                    dst[:, lo:m],
                    bass.AP(tensor=ext, offset=lo, ap=[[0, P], [1, m - lo]]),
                )
                lo = m

        # phase-A column data first (pn upper half feeds the first runs);
        # fine pieces so the first runs start as early as possible
        bcast(pn_sb, pn_ext, 4096, 6144, piece=512)
        bcast(pn_sb, pn_ext, 6144, 8192, piece=1024)
        rows_sb = constp.tile([P, NSLOT, 5], F32)
        nc.sync.dma_start(
            rows_sb[:], rows_ext[:, :].rearrange("p (r s) -> p r s", s=5)
        )
        phib = constp.tile([P, NSLOT, K], BF16)
        nc.sync.dma_start(
            phib[:], phib_ext[:, :].rearrange("p (r k) -> p r k", k=K)
        )
        for lo, hi in plan["t01_ranges"]:
            if hi > 4096:
                bcast(t01_sb, t01_ext, max(lo, 4096), hi)
        for lo, hi in plan["q_ranges"]:
            if hi > 4096:
                bcast(q_sb, q_ext, max(lo, 4096), hi)
        bcast(pn_sb, pn_ext, 2048, 4096)
        bcast(pn_sb, pn_ext, 0, 2048)
        for lo, hi in plan["t01_ranges"]:
            if lo < 4096:
                bcast(t01_sb, t01_ext, lo, min(hi, 4096))
        for lo, hi in plan["q_ranges"]:
            if lo < 4096:
                bcast(q_sb, q_ext, lo, min(hi, 4096))
        psi = constp.tile([K, N], F32)
        nc.sync.dma_start(psi[:], psi_ext[:, :])
        psiRA = constp.tile([K * 16, 256], F32)
        nc.sync.dma_start(psiRA[:], psiRA_ext[:, :])
        psiRB = constp.tile([K * 16, 128], F32)
        nc.sync.dma_start(psiRB[:], psiRB_ext[:, :])

        acc = sp.tile([K * 16, NACC], F32)
        nc.vector.memset(acc[:], 0.0)
        XsA = sp.tile([K, 4096], F32)
        XsB = sp.tile([K, 2048], F32)
        ttr_scr = sp.tile([K * 16, 256], F32)
        ttr_scr2 = sp.tile([K, CH], F32)

        chunk_tiles = {}

        def get_chunk(c):
            if c not in chunk_tiles:
                chunk_tiles[c] = pp.tile([K, CH], F32, tag="X", name=f"X{c}")
            return chunk_tiles[c]

        # ACT offload set: phase-A far runs of slots 1..3
        act_far = set()
        for k in (1, 2, 3):
            for kind, a, b in emitA[k]:
                if kind == "far":
                    act_far.add((k, a, b))

        def emit_run(k, kind, a, b, first_t, last_t):
            while b - a > 4096:
                emit_run(k, kind, a, a + 4096, first_t, last_t)
                a += 4096
            w = b - a
            v = vp.tile([P, 4096], BF16, tag="v")
            if kind == "cust":
                nc.vector._custom_dve(
                    ARL_PN,
                    out=v[:, :w],
                    in0=t01_sb[:, a:b],
                    in1=pn_sb[:, a:b],
                    s0=rows_sb[:, k, 0:1],
                    s1=rows_sb[:, k, 1:2],
                    imm2=0.1,
                )
            elif (k, a, b) in act_far:
                nc.scalar.activation(
                    v[:, :w],
                    pn_sb[:, a:b],
                    mybir.ActivationFunctionType.Relu,
                    bias=rows_sb[:, k, 4:5],
                    scale=1.0,
                )
            else:
                src = q_sb if kind == "mid" else pn_sb
                nc.vector.tensor_scalar(
                    v[:, :w],
                    src[:, a:b],
                    rows_sb[:, k, _KIND_SCAL[kind] : _KIND_SCAL[kind] + 1],
                    0.0,
                    AL.add,
                    AL.max,
                )
            c0, c1 = a // CH, (b - 1) // CH
            for c in range(c0, c1 + 1):
                lo, hi = max(a, c * CH), min(b, (c + 1) * CH)
                for m0 in range(lo, hi, 512):
                    m1 = min(m0 + 512, hi)
                    nc.tensor.matmul(
                        get_chunk(c)[:, m0 - c * CH : m1 - c * CH],
                        phib[:, k, :],
                        v[:, m0 - a : m1 - a],
                        start=(k == first_t[c]),
                        stop=(k == last_t[c]),
                    )

        def drain_copy(c, Xs, base):
            nc.scalar.copy(
                Xs[:, c * CH - base : (c + 1) * CH - base], get_chunk(c)[:]
            )

        def drain_direct(c, slot_idx):
            nc.vector._custom_dve(
                dve_ops.TENSOR_TENSOR_REDUCE,
                out=ttr_scr2[:],
                in0=get_chunk(c)[:],
                in1=psi[:, c * CH : (c + 1) * CH],
                s0=0.0,
                s1=1.0,
                accum_out=acc[0:K, slot_idx : slot_idx + 1],
            )

        # ---- phase A: columns [4096, 8192) ----
        drainedA = []
        for k in range(NSLOT):
            for kind, a, b in emitA[k]:
                emit_run(k, kind, a, b, firstA, lastA)
            for c in sorted(lastA):
                if lastA[c] == k:
                    drain_copy(c, XsA, PHASE_SPLIT)
                    drainedA.append(c)
        # reshape [7,4096] -> [112,256] (per-n DMAs); TTR emitted later so
        # the in-order DVE queue does not stall on the drain dependencies.
        xrA = sp.tile([K * 16, 256], F32)
        for n in range(K):
            nc.sync.dma_start(
                xrA[n * 16 : (n + 1) * 16, :],
                XsA[n : n + 1, :].rearrange("o (k f) -> o k f", k=16),
            )

        # ---- phase B: columns [0, 4096) ----
        xrB = sp.tile([K * 16, 128], F32)
        ttr_scrB = sp.tile([K * 16, 128], F32)
        for k in range(NSLOT):
            for kind, a, b in emitB[k]:
                emit_run(k, kind, a, b, firstB, lastB)
            for c in sorted(lastB):
                if lastB[c] == k:
                    if c <= 1:
                        drain_copy(c, XsB, 0)
                    else:
                        drain_direct(c, c)
            if k == 1:
                # xrB reshape can dispatch once chunks 0-1 are copied
                for n in range(K):
                    nc.sync.dma_start(
                        xrB[n * 16 : (n + 1) * 16, :],
                        XsB[n : n + 1, :].rearrange("o (k f) -> o k f", k=16),
                    )
            if k == 2:
                nc.vector._custom_dve(
                    dve_ops.TENSOR_TENSOR_REDUCE,
                    out=ttr_scr[:],
                    in0=xrA[:],
                    in1=psiRA[:],
                    s0=0.0,
                    s1=1.0,
                    accum_out=acc[:, 0:1],
                )
        nc.vector._custom_dve(
            dve_ops.TENSOR_TENSOR_REDUCE,
            out=ttr_scrB[:],
            in0=xrB[:],
            in1=psiRB[:],
            s0=0.0,
            s1=1.0,
            accum_out=acc[:, 1:2],
        )

        nc.sync.dma_start(
            out_ext[:].rearrange("(p c) -> p c", c=NACC), acc[:]
        )

    nc.compile()
    return nc


_NC_CACHE = {}


def _get_nc(plan, key):
    if key not in _NC_CACHE:
        _NC_CACHE[key] = _build_nc(plan)
    return _NC_CACHE[key]


def _exact_count(t: np.ndarray) -> int:
    n = t.shape[0]
    _, cnts = np.unique(t, return_counts=True)
    dup = int(sum(int(c) * (int(c) - 1) // 2 for c in cnts[cnts > 1]))
    return n * (n - 1) // 2 - dup


def _make_in_maps(predictions, targets, uncertainties):
    import ml_dtypes

    t = np.ascontiguousarray(np.asarray(targets, np.float32))
    p = np.ascontiguousarray(np.asarray(predictions, np.float32))
    u = np.ascontiguousarray(np.asarray(uncertainties, np.float32))
    order = np.argsort(t, kind="stable")
    ts, ps, us = t[order], p[order], u[order]
    ts64, ps64 = ts.astype(np.float64), ps.astype(np.float64)

    t01_h = (0.1 * ts64).astype(np.float16)
    pn_h = (-ps64).astype(np.float16)
    q_h = (0.1 * ts64 - ps64).astype(np.float16)

    # Psi[n, j] = x_j^n, x = u - 0.5
    x = us.astype(np.float64) - 0.5
    psi = np.stack([x**n for n in range(K)]).astype(np.float32)  # [K, N]
    psiRA = np.ascontiguousarray(
        psi[:, PHASE_SPLIT:].reshape(K, 16, 256).reshape(K * 16, 256)
    )
    psiRB = np.ascontiguousarray(
        psi[:, 0:2048].reshape(K, 16, 128).reshape(K * 16, 128)
    )

    # Phi[i, n] = sum_m A[m, n] x_i^m  (float64 -> bf16)
    xp = np.stack([x**m for m in range(K)])  # [K, N]
    phi = np.einsum("mn,mj->jn", _ACOEF, xp)  # [N, K]

    in_maps = []
    for c in range(NCORES):
        pos = (np.arange(NSLOT)[:, None] * 1024 + 8 * np.arange(P)[None, :] + c)
        # rows5[p, slot, s]
        rows5 = np.zeros((P, NSLOT, 5), np.float64)
        tp = ts64[pos]  # [slot, p]
        pp_ = ps64[pos]
        rows5[:, :, 0] = (0.1 * tp).T
        rows5[:, :, 1] = pp_.T
        rows5[:, :, 2] = (-(0.1 * tp - pp_)).T
        rows5[:, :, 3] = (pp_ + 0.01).T
        rows5[:, :, 4] = (pp_ + 0.1).T
        phib = phi[pos, :]  # [slot, p, K]
        phib = np.ascontiguousarray(np.transpose(phib, (1, 0, 2)))  # [p, slot, K]
        in_maps.append(
            {
                "t01col": t01_h,
                "pncol": pn_h,
                "qcol": q_h,
                "rows5": np.ascontiguousarray(
                    rows5.reshape(P, 5 * NSLOT)
                ).astype(np.float32),
                "phib": phib.reshape(P, NSLOT * K).astype(ml_dtypes.bfloat16),
                "psi": psi,
                "psiRA": psiRA,
                "psiRB": psiRB,
            }
        )
    plan = _make_plan(ts)
    return in_maps, t, plan


def _run_device(in_maps, plan, plan_key, trace=False, **kw):
    nc = _get_nc(plan, plan_key)
    return run_bass_kernel_spmd(
        nc, in_maps, core_ids=list(range(NCORES)), trace=trace, **kw
    )


def _plan_key(plan):
    return (
        tuple(tuple(r) for k in range(NSLOT) for r in plan["emitA"][k]),
        tuple(tuple(r) for k in range(NSLOT) for r in plan["emitB"][k]),
    )


def kernel(predictions, targets, uncertainties):
    in_maps, t, plan = _make_in_maps(predictions, targets, uncertainties)
    res = _run_device(in_maps, plan, _plan_key(plan))
    total = np.float64(0.0)
    for r in res.results:
        total += np.asarray(r["out"], np.float64).sum()
    count = _exact_count(t)
    return np.asarray(total / max(count, 1), dtype=np.float32)
